# revision 17
# baseline (speedup 1.0000x reference)
"""Multi-head attention + output proj + residual + global layernorm.

Shapes: B=8, S=1024, D=1024, H=16 heads, DK=64.
Strategy: data-parallel over batch -- each of the 8 NeuronCores computes one
full batch element (attention + out-proj + its own layernorm).  No
collectives needed since the layernorm normalizes over (S, D) per batch.

Per-core kernel (all on one core, batch b):
  phase 1: V = x @ Wv (all heads), Q^T/K^T per head-pair  (PE, f32r matmuls)
  phase 2: per head-pair: scores^T = K Q^T (transposed so the attn@V
           contraction dim lands on partitions), exp on ACT (scale=1/8
           folded in), mask multiply on DVE (bf16), ctx'^T = V'^T @ p^T with
           a ones-column appended to V so row 64 of ctx' is the softmax
           denominator; normalize via reciprocal + partition-broadcast.
  phase 3: out = ctxcat^T.T @ wo + x, then global layernorm via
           row sums/sumsq + cross-partition all-reduce.
"""

import os

import numpy as np
import ml_dtypes

P = 128
B, S, D, H = 8, 1024, 1024, 16
DK = D // H  # 64
NCH = D // P  # 8 contraction chunks
NT = S // P  # 8 tiles of 128 rows
SCH = 512  # free-dim chunk for matmuls
NSC = S // SCH  # 2

N_CORES = 8

_COMPILED = {}


def _emit_kernel(tc, aps, mm_dt):
    import concourse.bass as bass
    from concourse import mybir

    phases = os.environ.get("MHA_PHASES", "123")

    nc = tc.nc
    f32 = mybir.dt.float32
    bf16 = mybir.dt.bfloat16
    # matmul operand dtype for the f32-stored tensors (float32r runs the PE
    # at 1 cycle/row for free dims >= 256; plain float32 runs at 4)
    st_dt = mybir.dt.float32r if mm_dt == "f32r" else bf16

    def mmv(ap):
        return ap

    xT_d = aps["xT"]
    x_d = aps["x"]
    maskT_d = aps["maskT"]
    wq_d = aps["wq_all"]
    wk_d = aps["wk_all"]
    wv_d = aps["wv_all"]
    wo_d = aps["wo"]
    out_d = aps["out"]

    AF = mybir.ActivationFunctionType
    ALU = mybir.AluOpType

    # ---------------- pools ----------------
    persist = tc.alloc_tile_pool(name="persist", bufs=1)
    qkt_pool = tc.alloc_tile_pool(name="qkt", bufs=1)

    ctxcat = persist.tile([P, NCH, S], st_dt, tag="ctxcat")  # ctxcat^T chunks
    V_sb = persist.tile([P, NT, H, DK + 1], bf16, tag="v")  # V + ones col
    maskT_sb = persist.tile([P, NT, S], bf16, tag="maskT")
    stats_acc = persist.tile([P, 2 * NT], f32, tag="stats")
    ln_small = persist.tile([P, 10], f32, tag="lnsmall")
    nc.vector.memset(ln_small[:, 8:9], 1e-5)

    QT = qkt_pool.tile([P, NCH, S], st_dt, tag="qt")
    KT = qkt_pool.tile([P, NCH, S], st_dt, tag="kt")

    # ones column of V' (softmax denominator trick)
    nc.vector.memset(V_sb[:, :, :, DK : DK + 1], 1.0)

    # mask^T load (bf16, host-pretransposed)
    nc.sync.dma_start(
        out=maskT_sb[:], in_=maskT_d.rearrange("(t p) s -> p t s", p=P)
    )

    def load_chunk(stage_pool, dram_ap, c, out_ap):
        # DMA one [P, D] operand chunk; f32r needs a staged f32 DMA plus a
        # rounding convert (walrus requires f32r-consumed data to be rounded)
        if mm_dt == "bf16":
            nc.sync.dma_start(out=out_ap, in_=dram_ap[:, c, :])
        else:
            stg = stage_pool.tile([P, D], f32, tag="stage")
            nc.sync.dma_start(out=stg[:], in_=dram_ap[:, c, :])
            nc.scalar.copy(out=out_ap, in_=stg[:])

    # ================= phase 1: projections =================
    with (
        tc.tile_pool(name="ph1", bufs=1) as ph1,
        tc.tile_pool(name="wchunk", bufs=8) as wpool,
        tc.tile_pool(name="stage1", bufs=3) as stg1,
        tc.tile_pool(name="pj_psum", bufs=2, space="PSUM") as pj_psum,
    ):
        xT_sb = ph1.tile([P, NCH, S], st_dt, tag="xT")
        xT_r = xT_d.rearrange("(c p) s -> p c s", p=P)
        for c in range(NCH):
            load_chunk(stg1, xT_r, c, xT_sb[:, c, :])

        # ---- V for all heads ----
        wv_tiles = []
        for c in range(NCH):
            wt = wpool.tile([P, D], st_dt, tag="w")
            load_chunk(stg1, wv_d.rearrange("(c p) n -> p c n", p=P), c, wt[:])
            wv_tiles.append(wt)
        for t in range(NT):
            ps = pj_psum.tile([P, S], f32, tag="pj")
            for c in range(NCH):
                for j in range(NSC):
                    nc.tensor.matmul(
                        ps[:, j * SCH : (j + 1) * SCH],
                        lhsT=mmv(xT_sb[:, c, t * P : (t + 1) * P]),
                        rhs=mmv(wv_tiles[c][:, j * SCH : (j + 1) * SCH]),
                        start=(c == 0),
                        stop=(c == NCH - 1),
                    )
            nc.scalar.copy(
                out=V_sb[:, t, :, 0:DK],
                in_=ps[:].rearrange("p (h k) -> p h k", h=H),
            )

        # ---- Q^T and K^T for all head pairs ----
        for w_d, dst in ((wq_d, QT), (wk_d, KT)):
            w_tiles = []
            for c in range(NCH):
                wt = wpool.tile([P, D], st_dt, tag="w")
                load_chunk(stg1, w_d.rearrange("(c p) n -> p c n", p=P), c, wt[:])
                w_tiles.append(wt)
            for pr in range(NCH):  # head pair
                ps = pj_psum.tile([P, S], f32, tag="pj")
                for c in range(NCH):
                    for j in range(NSC):
                        nc.tensor.matmul(
                            ps[:, j * SCH : (j + 1) * SCH],
                            lhsT=mmv(w_tiles[c][:, pr * P : (pr + 1) * P]),
                            rhs=mmv(xT_sb[:, c, j * SCH : (j + 1) * SCH]),
                            start=(c == 0),
                            stop=(c == NCH - 1),
                        )
                nc.scalar.copy(out=dst[:, pr, :], in_=ps[:])

    if phases == "3":
        nc.vector.memset(ctxcat[:], 0.01)
    if phases == "1":
        with tc.tile_pool(name="dbg", bufs=2) as dbg:
            for t in range(NT):
                dt_ = dbg.tile([P, S], f32, tag="dbg")
                nc.vector.tensor_copy(out=dt_[:], in_=KT[:, t, :])
                nc.sync.dma_start(
                    out=out_d.rearrange("(t p) d -> p t d", p=P)[:, t, :], in_=dt_[:]
                )
        qkt_pool.release()
        persist.release()
        return

    # ================= phase 2: attention =================
    with (
        tc.tile_pool(name="p_pool", bufs=4) as p_pool,
        tc.tile_pool(name="rrow", bufs=2) as rrow_pool,
        tc.tile_pool(name="rb", bufs=2) as rb_pool,
        tc.tile_pool(name="sc_psum", bufs=2, space="PSUM") as sc_psum,
        tc.tile_pool(name="ctx_psum", bufs=2, space="PSUM") as ctx_psum,
    ):
        for pr in (range(NCH) if phases != "3" else []):
            ctxA = ctx_psum.tile([DK + 1, S], f32, tag="ctx")
            ctxB = ctx_psum.tile([DK + 1, S], f32, tag="ctx")
            ctxs = [ctxA, ctxB]
            for t in range(NT):
                for a in range(2):
                    sc = sc_psum.tile([P, S], f32, tag="sc")
                    lo, hi = a * DK, (a + 1) * DK
                    for j in range(NSC):
                        nc.tensor.matmul(
                            sc[:, j * SCH : (j + 1) * SCH],
                            lhsT=mmv(KT[lo:hi, pr, t * P : (t + 1) * P]),
                            rhs=mmv(QT[lo:hi, pr, j * SCH : (j + 1) * SCH]),
                            start=True,
                            stop=True,
                        )
                    praw = p_pool.tile([P, S], bf16, tag="p")
                    nc.scalar.activation(praw[:], sc[:], AF.Exp, scale=0.125)
                    pm = p_pool.tile([P, S], bf16, tag="p")
                    nc.vector.tensor_mul(pm[:], praw[:], maskT_sb[:, t, :])
                    h = 2 * pr + a
                    for j in range(NSC):
                        nc.tensor.matmul(
                            ctxs[a][:, j * SCH : (j + 1) * SCH],
                            lhsT=V_sb[:, t, h, :],
                            rhs=pm[:, j * SCH : (j + 1) * SCH],
                            start=(t == 0),
                            stop=(t == NT - 1),
                        )
            for a in range(2):
                # NB: reciprocal_approx_fast must not read PSUM directly --
                # it wedges the exec unit on hw. Copy the denominator row out.
                drow = rrow_pool.tile([1, S], f32, tag="drow")
                nc.vector.tensor_copy(out=drow[:], in_=ctxs[a][DK : DK + 1, :])
                rrow = rrow_pool.tile([1, S], f32, tag="rrow")
                nc.vector.reciprocal_approx_fast(out=rrow[:], in_=drow[:])
                rb = rb_pool.tile([DK, S], f32, tag="rb")
                nc.gpsimd.partition_broadcast(out_ap=rb[:], in_ap=rrow[:], channels=DK)
                nc.vector.tensor_mul(
                    ctxcat[a * DK : (a + 1) * DK, pr, :], ctxs[a][0:DK, :], rb[:]
                )

    qkt_pool.release()

    if phases == "12":
        with tc.tile_pool(name="dbg", bufs=2) as dbg:
            for t in range(NT):
                dt_ = dbg.tile([P, S], f32, tag="dbg")
                nc.vector.tensor_copy(out=dt_[:], in_=ctxcat[:, t, :])
                nc.sync.dma_start(
                    out=out_d.rearrange("(t p) d -> p t d", p=P)[:, t, :], in_=dt_[:]
                )
        persist.release()
        return

    # ================= phase 3: output projection + layernorm =================
    with (
        tc.tile_pool(name="wo_pool", bufs=8) as wo_pool,
        tc.tile_pool(name="x_pool", bufs=3) as x_pool,
        tc.tile_pool(name="res_pool", bufs=1) as res_pool,
        tc.tile_pool(name="scr_pool", bufs=2) as scr_pool,
        tc.tile_pool(name="out_pool", bufs=3) as o_pool,
        tc.tile_pool(name="stage3", bufs=3) as stg3,
        tc.tile_pool(name="op_psum", bufs=2, space="PSUM") as op_psum,
    ):
        res_sb = res_pool.tile([P, NT, D], f32, tag="res")
        wo_tiles = []
        for c in range(NCH):
            wt = wo_pool.tile([P, D], st_dt, tag="wo")
            load_chunk(stg3, wo_d.rearrange("(c p) n -> p c n", p=P), c, wt[:])
            wo_tiles.append(wt)
        for t in range(NT):
            ps = op_psum.tile([P, D], f32, tag="op")
            for c in range(NCH):
                for j in range(NSC):
                    nc.tensor.matmul(
                        ps[:, j * SCH : (j + 1) * SCH],
                        lhsT=mmv(ctxcat[:, c, t * P : (t + 1) * P]),
                        rhs=mmv(wo_tiles[c][:, j * SCH : (j + 1) * SCH]),
                        start=(c == 0),
                        stop=(c == NCH - 1),
                    )
            xt = x_pool.tile([P, D], f32, tag="x")
            nc.sync.dma_start(
                out=xt[:], in_=x_d.rearrange("(t p) d -> p t d", p=P)[:, t, :]
            )
            # res = psum + x, with free-dim row sums accumulated
            nc.vector.scalar_tensor_tensor(
                out=res_sb[:, t, :],
                in0=ps[:],
                scalar=1.0,
                in1=xt[:],
                op0=ALU.mult,
                op1=ALU.add,
                accum_out=stats_acc[:, t : t + 1],
            )
            scr = scr_pool.tile([P, D], f32, tag="scr")
            # NB: InstTensorTensorReduce wedges the exec unit on hw (flaky) --
            # use ACT Square with accum_out for the sum of squares instead.
            nc.scalar.activation(
                out=scr[:],
                in_=res_sb[:, t, :],
                func=AF.Square,
                accum_out=stats_acc[:, NT + t : NT + t + 1],
            )

        # global stats: reduce the 8 per-tile partials, then across partitions
        nc.vector.tensor_reduce(
            ln_small[:, 0:1], stats_acc[:, 0:NT], axis=mybir.AxisListType.X, op=ALU.add
        )
        nc.vector.tensor_reduce(
            ln_small[:, 1:2],
            stats_acc[:, NT : 2 * NT],
            axis=mybir.AxisListType.X,
            op=ALU.add,
        )
        from concourse import bass_isa

        nc.gpsimd.partition_all_reduce(
            ln_small[:, 2:4], ln_small[:, 0:2], channels=P, reduce_op=bass_isa.ReduceOp.add
        )
        inv_n = 1.0 / float(S * D)
        # mean, mean-of-squares
        nc.vector.tensor_scalar_mul(ln_small[:, 4:6], ln_small[:, 2:4], inv_n)
        # var = msq - mean^2
        nc.vector.tensor_mul(ln_small[:, 6:7], ln_small[:, 4:5], ln_small[:, 4:5])
        nc.vector.scalar_tensor_tensor(
            out=ln_small[:, 7:8],
            in0=ln_small[:, 5:6],
            scalar=1.0,
            in1=ln_small[:, 6:7],
            op0=ALU.mult,
            op1=ALU.subtract,
        )
        # sd = sqrt(var + eps); rstd = 1/sd
        nc.scalar.activation(
            ln_small[:, 6:7], ln_small[:, 7:8], AF.Sqrt, bias=ln_small[:, 8:9]
        )
        nc.vector.reciprocal(ln_small[:, 7:8], ln_small[:, 6:7])

        for t in range(NT):
            ot = o_pool.tile([P, D], f32, tag="o")
            nc.vector.tensor_scalar(
                out=ot[:],
                in0=res_sb[:, t, :],
                scalar1=ln_small[:, 4:5],
                scalar2=ln_small[:, 7:8],
                op0=ALU.subtract,
                op1=ALU.mult,
            )
            nc.sync.dma_start(
                out=out_d.rearrange("(t p) d -> p t d", p=P)[:, t, :], in_=ot[:]
            )

    persist.release()


def _build(mm_dt):
    import concourse.tile as tile
    from concourse import bacc, mybir

    f32 = mybir.dt.float32
    bf16 = mybir.dt.bfloat16
    st_np = f32 if mm_dt == "f32r" else bf16

    nc = bacc.Bacc(
        "TRN2", target_bir_lowering=False, debug=False, num_devices=N_CORES
    )
    aps = {}
    aps["xT"] = nc.dram_tensor("xT", [D, S], st_np, kind="ExternalInput").ap()
    aps["x"] = nc.dram_tensor("x", [S, D], f32, kind="ExternalInput").ap()
    aps["maskT"] = nc.dram_tensor("maskT", [S, S], bf16, kind="ExternalInput").ap()
    aps["wq_all"] = nc.dram_tensor("wq_all", [D, D], st_np, kind="ExternalInput").ap()
    aps["wk_all"] = nc.dram_tensor("wk_all", [D, D], st_np, kind="ExternalInput").ap()
    aps["wv_all"] = nc.dram_tensor("wv_all", [D, D], st_np, kind="ExternalInput").ap()
    aps["wo"] = nc.dram_tensor("wo", [D, D], st_np, kind="ExternalInput").ap()
    aps["out"] = nc.dram_tensor("out", [S, D], f32, kind="ExternalOutput").ap()

    with tile.TileContext(nc) as tc:
        _emit_kernel(tc, aps, mm_dt)
    nc.compile()
    return nc


def _get_program(mm_dt):
    if mm_dt not in _COMPILED:
        _COMPILED[mm_dt] = _build(mm_dt)
    return _COMPILED[mm_dt]


def kernel(mask, x_key_value, wq, wk, wv, wo):
    from concourse.bass_utils import run_bass_kernel_spmd

    mm_dt = os.environ.get("MHA_MM_DT", "f32r")
    st_np = np.float32 if mm_dt == "f32r" else ml_dtypes.bfloat16

    mask = np.asarray(mask)
    x = np.ascontiguousarray(np.asarray(x_key_value, dtype=np.float32))
    wq = np.asarray(wq, dtype=np.float32)
    wk = np.asarray(wk, dtype=np.float32)
    wv = np.asarray(wv, dtype=np.float32)
    wo = np.asarray(wo, dtype=np.float32)

    # host-side layout prep (shared across cores)
    wq_all = np.ascontiguousarray(wq.transpose(1, 0, 2).reshape(D, D).astype(st_np))
    wk_all = np.ascontiguousarray(wk.transpose(1, 0, 2).reshape(D, D).astype(st_np))
    wv_all = np.ascontiguousarray(wv.transpose(1, 0, 2).reshape(D, D).astype(st_np))
    wo_p = np.ascontiguousarray(wo.astype(st_np))

    in_maps = []
    for b in range(N_CORES):
        in_maps.append(
            {
                "xT": np.ascontiguousarray(x[b].T.astype(st_np)),
                "x": x[b],
                "maskT": np.ascontiguousarray(
                    mask[b].T.astype(ml_dtypes.bfloat16)
                ),
                "wq_all": wq_all,
                "wk_all": wk_all,
                "wv_all": wv_all,
                "wo": wo_p,
            }
        )

    nc = _get_program(mm_dt)
    trace = os.environ.get("MHA_TRACE", "0") == "1"
    res = run_bass_kernel_spmd(
        nc, in_maps, list(range(N_CORES)), trace=trace
    )
    if trace:
        print("HW exec time:", res.exec_time_ns, "ns")
        print("mean exec time:", res.mean_exec_time_ns, "ns")
    out = np.stack([res.results[i]["out"] for i in range(N_CORES)], axis=0)
    return out.astype(np.float32)
